# revision 6
# baseline (speedup 1.0000x reference)
"""Trainium2 Bass kernel for nn_CausalGP: GP posterior mean + variance.

Math (per batch b):
    XA   = concat([X[b], A[b]])                       [M, D], D = P+1 = 257
    Q    = exp(-0.5 * ||XA_m - XA_train_t||^2)        [M, N]   (RBF cross-kernel)
    f_loc[m] = sum_t Q[m,t] * alpha[t]
    f_var[m] = 1 - sum_{t,n} Q[m,t] K_inv[t,n] Q[m,n]
(only the diagonal of the covariance is ever needed -> never materialize [M,M]).

Sharding: pure data-parallel over B (8 batches -> 8 cores). XA_train, alpha,
K_inv replicated.

Key algebraic cut: the quadratic form d[m] = p^T K p (p = Q^T[:, m]) only
depends on the symmetric part S = (K + K^T)/2.  Tiled over 128-blocks,
    d = sum_J p_J . w*_J,   w*_J = S_JJ p_J + sum_{I>J} 2 S_IJ^T p_I
so only lower-triangular (I >= J) tiles of S participate: 528 tile-matmuls
instead of 1024.  The host packs those tiles (with the x2 / x0.5 coefficients
and a global x64 fp8-range scale folded in) into DoubleRow pair-slots; odd
tails are zero-padded against a zeroed pt guard tile.

Device layout (per core):
  PT[t, m] = Q^T via PE matmul: fp8 DoubleRow over the 256 X-dims plus a bf16
  2-row matmul for the A-cross term and the -0.5||x_m||^2 row; per-partition
  exp bias carries -0.5||xa_t||^2 (computed on host, fp32).
  The loop runs J descending, interleaving PT tile production one step ahead
  of the ST consumer group so TensorE never waits on the exp activations; the
  1-slot group J=30 is held back to the very end so the serial drain after
  the last matmul is minimal.
  f_var accumulates sum_n w*[n,m]*PT[n,m] on VectorE; final cross-partition
  reduction via a ones-vector matmul, descaled by 1/64 in the output
  activation.  f_loc = sum_J alpha_J (x) PT_J runs entirely on VectorE
  (scalar_tensor_tensor with fp32 alpha), deferred to the back half of the
  loop where TensorE steps are long, and finalized before the last ST groups
  so only f_var sits in the tail.
  All K tiles live resident in SBUF (8.9 MB), DMA'd in 16 chunks; a burst of
  small self-matmuls at kernel start warms the PE HAM clock gate while the
  input DMAs stream.
"""

import numpy as np
import ml_dtypes

# ---- problem constants (hardcoded per contract) ----
B, M, P, N = 8, 1024, 256, 4096
D = P + 1          # 257 dims of XA
NT = N // 128      # 32 tiles of train points
MH = M // 512      # 2 moving-operand halves
KSCALE = 64.0      # fp8-range scale folded into the S tiles

GP_ADD = True      # accv adds on GpSimd instead of VectorE

# DoubleRow slot table for the triangular ST stage (slot storage order is
# J descending).  Slot s of group J pairs contraction tiles (J+2s, J+2s+1).
_SJ = {J: (NT - J + 1) // 2 for J in range(NT)}
_OFF = {}
_cur = 0
for _J in range(NT - 1, -1, -1):
    _OFF[_J] = _cur
    _cur += _SJ[_J]
NSLOT = _cur       # 272

# ST group emission order: J=30 (1 slot) held back for a minimal tail
_ST_ORDER = [31] + list(range(29, -1, -1)) + [30]

_CACHE = {}


def _build_program():
    import concourse.bass as bass
    import concourse.tile as tile
    from concourse import bacc, mybir
    from concourse.bass import ts

    bf16 = mybir.dt.bfloat16
    fp8 = mybir.dt.float8e4
    f32 = mybir.dt.float32
    FT = mybir.ActivationFunctionType
    OP = mybir.AluOpType
    DR = mybir.MatmulPerfMode.DoubleRow

    nc = bacc.Bacc(None, target_bir_lowering=False)

    # xa01: [d_in(128), chunk(2), t] = XA_train[t, chunk*128 + d_in]
    xa01 = nc.dram_tensor("xa01", [128, 2, N], fp8, kind="ExternalInput")
    # xa2: [A_train col; ones] extra contraction rows (bf16 for accuracy)
    xa2 = nc.dram_tensor("xa2", [2, N], bf16, kind="ExternalInput")
    # xb01: [d_in(128), chunk(2), m] = X_b[m, chunk*128 + d_in]
    xb01_h = nc.dram_tensor("xb01", [128, 2, M], fp8, kind="ExternalInput")
    # xb2: [A_b row; -0.5*||xa_m||^2 row]
    xb2_h = nc.dram_tensor("xb2", [2, M], bf16, kind="ExternalInput")
    # z2negh: [t_in(128), ntile] = -0.5*||XA_train_t||^2 (exp bias, fp32)
    z2negh = nc.dram_tensor("z2negh", [128, NT], f32, kind="ExternalInput")
    # ktri: packed triangular DoubleRow slots [t_in(128), slot, pair(2), n_in(128)]
    ktri = nc.dram_tensor("ktri", [128, NSLOT, 2, 128], fp8, kind="ExternalInput")
    alphaf = nc.dram_tensor("alphaf", [128, NT], f32, kind="ExternalInput")
    out = nc.dram_tensor("out", [2, M], f32, kind="ExternalOutput")

    with tile.TileContext(nc) as tc:
        with (
            tc.tile_pool(name="singles", bufs=1) as singles,
            tc.tile_pool(name="tmppool", bufs=12) as tmppool,
            tc.tile_pool(name="psum", bufs=8, space="PSUM") as psum,
        ):
            # ---------------- resident tiles ----------------
            wtile = singles.tile([128, 128], fp8)    # HAM warmup operand
            xt01 = singles.tile([128, 2, N], fp8)
            xt2 = singles.tile([2, N], bf16)
            xb01 = singles.tile([128, 2, M], fp8)
            xb2 = singles.tile([2, M], bf16)
            alpha_sb = singles.tile([128, NT], f32)
            z2neg = singles.tile([128, NT], f32)
            ones_sb = singles.tile([128, 1], f32)
            ksb = singles.tile([128, NSLOT, 2, 128], fp8)
            pt = singles.tile([128, NT + 1, M], fp8)  # Q^T + zero guard tile
            accv = singles.tile([128, M], f32)       # partial diag sums over n
            facc = singles.tile([128, M], f32)       # partial f_loc sums
            floc_sb = singles.tile([1, M], f32)
            fvar_sb = singles.tile([1, M], f32)

            nc.vector.memset(wtile, 0.0)     # first: unblocks the HAM warmup

            nc.sync.dma_start(out=xt01, in_=xa01[:, :, :])
            nc.sync.dma_start(out=xb01, in_=xb01_h[:, :, :])
            nc.sync.dma_start(out=xt2, in_=xa2[:, :])
            nc.sync.dma_start(out=xb2, in_=xb2_h[:, :])
            nc.sync.dma_start(out=alpha_sb, in_=alphaf[:, :])
            nc.sync.dma_start(out=z2neg, in_=z2negh[:, :])
            NCH, CSZ = 16, NSLOT // 16
            for c in range(NCH):
                nc.sync.dma_start(out=ksb[:, c * CSZ:(c + 1) * CSZ],
                                  in_=ktri[:, c * CSZ:(c + 1) * CSZ])

            nc.vector.memset(ones_sb, 1.0)
            nc.vector.memset(pt[:, NT, :], 0.0)      # DR zero-pad guard
            nc.vector.memset(accv, 0.0)
            nc.vector.memset(facc, 0.0)

            # ---------------- HAM warmup: keep PE busy under the input DMAs
            # (operand content is irrelevant -- wps is never read)
            wps = psum.tile([128, 512], f32, tag="big", name="warm")
            for _ in range(52):
                nc.tensor.matmul(wps[:, 0:128], wtile, wtile,
                                 start=True, stop=True)

            floc_pending = []
            floc_done = False

            # ---------------- interleaved PT producer / ST consumer ----
            for k in range(NT + 1):
                if k < NT:
                    i = NT - 1 - k
                    # PT(i): arg = XA_train_i @ XA^T - 0.5||xa_m||^2 (rank-2
                    # rows in bf16), exp bias carries -0.5||xa_t||^2
                    pps = [psum.tile([128, 512], f32, tag="big",
                                     name=f"pp{i}_{h}") for h in range(MH)]
                    for mh in range(MH):
                        nc.tensor.matmul(pps[mh], xt01[:, :, ts(i, 128)],
                                         xb01[:, :, ts(mh, 512)],
                                         start=True, stop=False, perf_mode=DR)
                    for mh in range(MH):
                        nc.tensor.matmul(pps[mh], xt2[:, ts(i, 128)],
                                         xb2[:, ts(mh, 512)],
                                         start=False, stop=True)
                    for mh in range(MH):
                        nc.scalar.activation(
                            out=pt[:, i, ts(mh, 512)], in_=pps[mh], func=FT.Exp,
                            bias=z2neg[:, i:i + 1], scale=1.0,
                        )
                    floc_pending.append(i)
                if k == 0:
                    continue
                # ST(J): w*_J = sum_s kt_s^T @ pt[pair_s]  (triangular, scaled)
                J = _ST_ORDER[k - 1]
                sJ = _SJ[J]
                sts = [psum.tile([128, 512], f32, tag="big",
                                 name=f"st{J}_{h}") for h in range(MH)]
                for s in range(sJ):
                    for mh in range(MH):
                        nc.tensor.matmul(
                            sts[mh], ksb[:, _OFF[J] + s],
                            pt[:, J + 2 * s:J + 2 * s + 2, ts(mh, 512)],
                            start=(s == 0), stop=(s == sJ - 1), perf_mode=DR,
                        )
                # accv += pt_J * w*_J  (diag contribution of this n-tile row)
                add_eng = nc.gpsimd if (GP_ADD and k <= NT - 1) else nc.vector
                for mh in range(MH):
                    tmp = tmppool.tile([128, 512], f32)
                    nc.vector.tensor_mul(tmp, sts[mh], pt[:, J, ts(mh, 512)])
                    add_eng.tensor_add(accv[:, ts(mh, 512)],
                                       accv[:, ts(mh, 512)], tmp)
                # deferred f_loc accumulation: facc += alpha_J (x) pt_J, run
                # in the back half where TensorE steps are long
                if k >= 8:
                    for i2 in floc_pending[:3]:
                        for mh in range(MH):
                            nc.vector.scalar_tensor_tensor(
                                out=facc[:, ts(mh, 512)],
                                in0=pt[:, i2, ts(mh, 512)],
                                scalar=alpha_sb[:, i2:i2 + 1],
                                in1=facc[:, ts(mh, 512)],
                                op0=OP.mult, op1=OP.add,
                            )
                    floc_pending = floc_pending[3:]
                # finalize f_loc as soon as every tile is folded in, so it
                # overlaps the remaining ST groups
                if not floc_pending and not floc_done and k >= 24:
                    floc_done = True
                    for mh in range(MH):
                        qf = psum.tile([1, 512], f32, tag="big", name=f"qf{mh}")
                        nc.tensor.matmul(qf, ones_sb, facc[:, ts(mh, 512)],
                                         start=True, stop=True)
                        nc.scalar.copy(floc_sb[0:1, ts(mh, 512)], qf)
                    nc.sync.dma_start(out=out[0:1, :], in_=floc_sb)

            assert floc_done and not floc_pending

            # ---------------- f_var = 1 - (ones^T @ accv) / KSCALE ----------
            for mh in range(MH):
                q = psum.tile([1, 512], f32, tag="big", name=f"q{mh}")
                nc.tensor.matmul(q, ones_sb, accv[:, ts(mh, 512)],
                                 start=True, stop=True)
                nc.scalar.activation(
                    out=fvar_sb[0:1, ts(mh, 512)], in_=q, func=FT.Identity,
                    scale=-1.0 / KSCALE, bias=1.0,
                )
            nc.sync.dma_start(out=out[1:2, :], in_=fvar_sb)

    nc.compile()
    return nc


def _host_inputs(X, A, XA_train, alpha, K_inv):
    nd = ml_dtypes.float8_e4m3
    bf = ml_dtypes.bfloat16

    XT = XA_train.T.astype(np.float32)                      # [D, N]
    xa01 = np.ascontiguousarray(
        XT[:256].reshape(2, 128, N).transpose(1, 0, 2)).astype(nd)  # [128, 2, N]
    xa2 = np.empty((2, N), dtype=bf)
    xa2[0] = XT[256].astype(bf)
    xa2[1] = np.ones(N, dtype=bf)

    z2 = -0.5 * np.sum(XA_train.astype(np.float32) ** 2, axis=1)   # [N]
    z2negh = np.ascontiguousarray(z2.reshape(NT, 128).T)           # [128, NT]

    # triangular DoubleRow slot packing of T = K + K^T (x64 fp8-range scale;
    # diagonal tiles carry 0.5x, off-diagonal 1x == the symmetry 2x)
    T = (K_inv + K_inv.T).astype(np.float32)
    ktri = np.zeros((128, NSLOT, 2, 128), dtype=np.float32)
    for J in range(NT):
        for s in range(_SJ[J]):
            for c in range(2):
                I = J + 2 * s + c
                if I >= NT:
                    continue
                w = 0.5 * KSCALE if I == J else KSCALE
                ktri[:, _OFF[J] + s, c, :] = (
                    w * T[I * 128:(I + 1) * 128, J * 128:(J + 1) * 128])
    ktri = ktri.astype(nd)

    alphaf = np.ascontiguousarray(
        alpha.astype(np.float32).reshape(NT, 128).T)        # [128, NT]

    shared = {"xa01": xa01, "xa2": xa2, "z2negh": z2negh, "ktri": ktri,
              "alphaf": alphaf}

    in_maps = []
    for b in range(B):
        Xb = X[b].astype(np.float32)                        # [M, P]
        xb01 = np.ascontiguousarray(
            Xb.T.reshape(2, 128, M).transpose(1, 0, 2)).astype(nd)  # [128, 2, M]
        ab = A[b].astype(np.float32)
        xb2 = np.empty((2, M), dtype=bf)
        xb2[0] = ab.astype(bf)
        xb2[1] = (-0.5 * (np.sum(Xb * Xb, axis=1) + ab)).astype(bf)
        in_maps.append({**shared, "xb01": xb01, "xb2": xb2})
    return in_maps


def _run(X, A, XA_train, alpha, K_inv, trace=False, tmpdir=None):
    from concourse.bass_utils import run_bass_kernel_spmd

    if "nc" not in _CACHE:
        _CACHE["nc"] = _build_program()
    nc = _CACHE["nc"]

    in_maps = _host_inputs(X, A, XA_train, alpha, K_inv)
    kw = {}
    if trace:
        kw = dict(trace=True, tmpdir=tmpdir)
    res = run_bass_kernel_spmd(nc, in_maps, core_ids=list(range(B)), **kw)

    f_loc = np.stack([res.results[b]["out"][0] for b in range(B)]).astype(np.float32)
    f_var = np.stack([res.results[b]["out"][1] for b in range(B)]).astype(np.float32)
    return (f_loc, f_var), res


def kernel(X, A, XA_train, alpha, K_inv):
    (f_loc, f_var), _ = _run(
        np.asarray(X), np.asarray(A), np.asarray(XA_train),
        np.asarray(alpha), np.asarray(K_inv),
    )
    return f_loc, f_var


# revision 7
# speedup vs baseline: 1.0551x; 1.0551x over previous
"""Trainium2 Bass kernel for nn_CausalGP: GP posterior mean + variance.

Math (per batch b):
    XA   = concat([X[b], A[b]])                       [M, D], D = P+1 = 257
    Q    = exp(-0.5 * ||XA_m - XA_train_t||^2)        [M, N]   (RBF cross-kernel)
    f_loc[m] = sum_t Q[m,t] * alpha[t]
    f_var[m] = 1 - sum_{t,n} Q[m,t] K_inv[t,n] Q[m,n]
(only the diagonal of the covariance is ever needed -> never materialize [M,M]).

Sharding: pure data-parallel over B (8 batches -> 8 cores). XA_train, alpha,
K_inv replicated.

Key algebraic cut: the quadratic form d[m] = p^T K p (p = Q^T[:, m]) only
depends on the symmetric part S = (K + K^T)/2.  Tiled over 128-blocks,
    d = sum_J p_J . w*_J,   w*_J = S_JJ p_J + sum_{I>J} 2 S_IJ^T p_I
so only lower-triangular (I >= J) tiles of S participate: 528 tile-matmuls
instead of 1024.  The host packs those tiles (with the x2 / x0.5 coefficients
and a global x64 fp8-range scale folded in) into DoubleRow pair-slots; odd
tails are zero-padded against a zeroed pt guard tile.

Device layout (per core):
  PT[t, m] = Q^T via PE matmul: fp8 DoubleRow over the 256 X-dims plus a bf16
  2-row matmul for the A-cross term and the -0.5||x_m||^2 row; per-partition
  exp bias carries -0.5||xa_t||^2 (computed on host, fp32).
  The loop runs J descending, interleaving PT tile production one step ahead
  of the ST consumer group so TensorE never waits on the exp activations; the
  1-slot group J=30 is held back to the very end so the serial drain after
  the last matmul is minimal.
  f_var accumulates sum_n w*[n,m]*PT[n,m] on VectorE; final cross-partition
  reduction via a ones-vector matmul, descaled by 1/64 in the output
  activation.  f_loc = sum_J alpha_J (x) PT_J runs entirely on VectorE
  (scalar_tensor_tensor with fp32 alpha), deferred to the back half of the
  loop where TensorE steps are long, and finalized before the last ST groups
  so only f_var sits in the tail.
  All K tiles live resident in SBUF (8.9 MB), DMA'd in 16 chunks; a burst of
  small self-matmuls at kernel start warms the PE HAM clock gate while the
  input DMAs stream.
"""

import numpy as np
import ml_dtypes

# ---- problem constants (hardcoded per contract) ----
B, M, P, N = 8, 1024, 256, 4096
D = P + 1          # 257 dims of XA
NT = N // 128      # 32 tiles of train points
MH = M // 512      # 2 moving-operand halves
KSCALE = 64.0      # fp8-range scale folded into the S tiles

GP_ADD = False     # accv adds on GpSimd instead of VectorE

# DoubleRow slot table for the triangular ST stage (slot storage order is
# J descending).  Slot s of group J pairs contraction tiles (J+2s, J+2s+1).
_SJ = {J: (NT - J + 1) // 2 for J in range(NT)}
_OFF = {}
_cur = 0
for _J in range(NT - 1, -1, -1):
    _OFF[_J] = _cur
    _cur += _SJ[_J]
NSLOT = _cur       # 272

# ST group emission order: J=29 (2 slots) held back for a minimal tail.
# PT tiles 31 and 30 are both produced at step 0 so every ST group's newest
# pt tile is at least one full step old (its exp is off the critical path).
_ST_ORDER = [31, 30] + list(range(28, -1, -1)) + [29]

_CACHE = {}


def _build_program():
    import concourse.bass as bass
    import concourse.tile as tile
    from concourse import bacc, mybir
    from concourse.bass import ts

    bf16 = mybir.dt.bfloat16
    fp8 = mybir.dt.float8e4
    f32 = mybir.dt.float32
    FT = mybir.ActivationFunctionType
    OP = mybir.AluOpType
    DR = mybir.MatmulPerfMode.DoubleRow

    nc = bacc.Bacc(None, target_bir_lowering=False)

    # xa01: [d_in(128), chunk(2), t] = XA_train[t, chunk*128 + d_in]
    xa01 = nc.dram_tensor("xa01", [128, 2, N], fp8, kind="ExternalInput")
    # xa2: [A_train col; ones] extra contraction rows (bf16 for accuracy)
    xa2 = nc.dram_tensor("xa2", [2, N], bf16, kind="ExternalInput")
    # xb01: [d_in(128), chunk(2), m] = X_b[m, chunk*128 + d_in]
    xb01_h = nc.dram_tensor("xb01", [128, 2, M], fp8, kind="ExternalInput")
    # xb2: [A_b row; -0.5*||xa_m||^2 row]
    xb2_h = nc.dram_tensor("xb2", [2, M], bf16, kind="ExternalInput")
    # z2negh: [t_in(128), ntile] = -0.5*||XA_train_t||^2 (exp bias, fp32)
    z2negh = nc.dram_tensor("z2negh", [128, NT], f32, kind="ExternalInput")
    # ktri: packed triangular DoubleRow slots [t_in(128), slot, pair(2), n_in(128)]
    ktri = nc.dram_tensor("ktri", [128, NSLOT, 2, 128], fp8, kind="ExternalInput")
    alphaf = nc.dram_tensor("alphaf", [128, NT], f32, kind="ExternalInput")
    out = nc.dram_tensor("out", [2, M], f32, kind="ExternalOutput")

    with tile.TileContext(nc) as tc:
        with (
            tc.tile_pool(name="singles", bufs=1) as singles,
            tc.tile_pool(name="tmppool", bufs=12) as tmppool,
            tc.tile_pool(name="psum", bufs=8, space="PSUM") as psum,
        ):
            # ---------------- resident tiles ----------------
            wtile = singles.tile([128, 128], fp8)    # HAM warmup operand
            xt01 = singles.tile([128, 2, N], fp8)
            xt2 = singles.tile([2, N], bf16)
            xb01 = singles.tile([128, 2, M], fp8)
            xb2 = singles.tile([2, M], bf16)
            alpha_sb = singles.tile([128, NT], f32)
            z2neg = singles.tile([128, NT], f32)
            ones_sb = singles.tile([128, 1], f32)
            ksb = singles.tile([128, NSLOT, 2, 128], fp8)
            pt = singles.tile([128, NT + 1, M], fp8)  # Q^T + zero guard tile
            accv = singles.tile([128, M], f32)       # partial diag sums over n
            facc = singles.tile([128, M], f32)       # partial f_loc sums
            floc_sb = singles.tile([1, M], f32)
            fvar_sb = singles.tile([1, M], f32)

            nc.vector.memset(wtile, 0.0)     # first: unblocks the HAM warmup

            nc.sync.dma_start(out=xt01, in_=xa01[:, :, :])
            nc.sync.dma_start(out=xb01, in_=xb01_h[:, :, :])
            nc.sync.dma_start(out=xt2, in_=xa2[:, :])
            nc.sync.dma_start(out=xb2, in_=xb2_h[:, :])
            nc.sync.dma_start(out=alpha_sb, in_=alphaf[:, :])
            nc.sync.dma_start(out=z2neg, in_=z2negh[:, :])
            NCH, CSZ = 16, NSLOT // 16
            for c in range(NCH):
                nc.sync.dma_start(out=ksb[:, c * CSZ:(c + 1) * CSZ],
                                  in_=ktri[:, c * CSZ:(c + 1) * CSZ])

            nc.vector.memset(ones_sb, 1.0)
            nc.vector.memset(pt[:, NT, :], 0.0)      # DR zero-pad guard
            nc.vector.memset(accv, 0.0)
            nc.vector.memset(facc, 0.0)

            # ---------------- HAM warmup: keep PE busy under the input DMAs
            # (operand content is irrelevant -- wps is never read)
            wps = psum.tile([128, 512], f32, tag="big", name="warm")
            for _ in range(52):
                nc.tensor.matmul(wps[:, 0:128], wtile, wtile,
                                 start=True, stop=True)

            floc_pending = []
            floc_done = False

            # ---------------- interleaved PT producer / ST consumer ----
            for k in range(NT + 1):
                pt_make = [31, 30] if k == 0 else (
                    [30 - k] if k <= 30 else [])
                for i in pt_make:
                    # PT(i): arg = XA_train_i @ XA^T - 0.5||xa_m||^2 (rank-2
                    # rows in bf16), exp bias carries -0.5||xa_t||^2
                    pps = [psum.tile([128, 512], f32, tag="big",
                                     name=f"pp{i}_{h}") for h in range(MH)]
                    for mh in range(MH):
                        nc.tensor.matmul(pps[mh], xt01[:, :, ts(i, 128)],
                                         xb01[:, :, ts(mh, 512)],
                                         start=True, stop=False, perf_mode=DR)
                    for mh in range(MH):
                        nc.tensor.matmul(pps[mh], xt2[:, ts(i, 128)],
                                         xb2[:, ts(mh, 512)],
                                         start=False, stop=True)
                    for mh in range(MH):
                        nc.scalar.activation(
                            out=pt[:, i, ts(mh, 512)], in_=pps[mh], func=FT.Exp,
                            bias=z2neg[:, i:i + 1], scale=1.0,
                        )
                    floc_pending.append(i)
                if k == 0:
                    continue
                # ST(J): w*_J = sum_s kt_s^T @ pt[pair_s]  (triangular, scaled)
                J = _ST_ORDER[k - 1]
                sJ = _SJ[J]
                sts = [psum.tile([128, 512], f32, tag="big",
                                 name=f"st{J}_{h}") for h in range(MH)]
                for s in range(sJ):
                    for mh in range(MH):
                        nc.tensor.matmul(
                            sts[mh], ksb[:, _OFF[J] + s],
                            pt[:, J + 2 * s:J + 2 * s + 2, ts(mh, 512)],
                            start=(s == 0), stop=(s == sJ - 1), perf_mode=DR,
                        )
                # accv += pt_J * w*_J  (diag contribution of this n-tile row)
                add_eng = nc.gpsimd if (GP_ADD and k <= NT - 1) else nc.vector
                for mh in range(MH):
                    tmp = tmppool.tile([128, 512], f32)
                    nc.vector.tensor_mul(tmp, sts[mh], pt[:, J, ts(mh, 512)])
                    add_eng.tensor_add(accv[:, ts(mh, 512)],
                                       accv[:, ts(mh, 512)], tmp)
                # deferred f_loc accumulation: facc += alpha_J (x) pt_J, run
                # in the back half where TensorE steps are long
                if k >= 12:
                    for i2 in floc_pending[:3]:
                        for mh in range(MH):
                            nc.vector.scalar_tensor_tensor(
                                out=facc[:, ts(mh, 512)],
                                in0=pt[:, i2, ts(mh, 512)],
                                scalar=alpha_sb[:, i2:i2 + 1],
                                in1=facc[:, ts(mh, 512)],
                                op0=OP.mult, op1=OP.add,
                            )
                    floc_pending = floc_pending[3:]
                # finalize f_loc as soon as every tile is folded in, so it
                # overlaps the remaining ST groups
                if not floc_pending and not floc_done and k >= 24:
                    floc_done = True
                    for mh in range(MH):
                        qf = psum.tile([1, 512], f32, tag="big", name=f"qf{mh}")
                        nc.tensor.matmul(qf, ones_sb, facc[:, ts(mh, 512)],
                                         start=True, stop=True)
                        nc.scalar.copy(floc_sb[0:1, ts(mh, 512)], qf)
                    nc.sync.dma_start(out=out[0:1, :], in_=floc_sb)

            assert floc_done and not floc_pending

            # ---------------- f_var = 1 - (ones^T @ accv) / KSCALE ----------
            for mh in range(MH):
                q = psum.tile([1, 512], f32, tag="big", name=f"q{mh}")
                nc.tensor.matmul(q, ones_sb, accv[:, ts(mh, 512)],
                                 start=True, stop=True)
                nc.scalar.activation(
                    out=fvar_sb[0:1, ts(mh, 512)], in_=q, func=FT.Identity,
                    scale=-1.0 / KSCALE, bias=1.0,
                )
            nc.sync.dma_start(out=out[1:2, :], in_=fvar_sb)

    nc.compile()
    return nc


def _host_inputs(X, A, XA_train, alpha, K_inv):
    nd = ml_dtypes.float8_e4m3
    bf = ml_dtypes.bfloat16

    XT = XA_train.T.astype(np.float32)                      # [D, N]
    xa01 = np.ascontiguousarray(
        XT[:256].reshape(2, 128, N).transpose(1, 0, 2)).astype(nd)  # [128, 2, N]
    xa2 = np.empty((2, N), dtype=bf)
    xa2[0] = XT[256].astype(bf)
    xa2[1] = np.ones(N, dtype=bf)

    z2 = -0.5 * np.sum(XA_train.astype(np.float32) ** 2, axis=1)   # [N]
    z2negh = np.ascontiguousarray(z2.reshape(NT, 128).T)           # [128, NT]

    # triangular DoubleRow slot packing of T = K + K^T (x64 fp8-range scale;
    # diagonal tiles carry 0.5x, off-diagonal 1x == the symmetry 2x)
    T = (K_inv + K_inv.T).astype(np.float32)
    ktri = np.zeros((128, NSLOT, 2, 128), dtype=np.float32)
    for J in range(NT):
        for s in range(_SJ[J]):
            for c in range(2):
                I = J + 2 * s + c
                if I >= NT:
                    continue
                w = 0.5 * KSCALE if I == J else KSCALE
                ktri[:, _OFF[J] + s, c, :] = (
                    w * T[I * 128:(I + 1) * 128, J * 128:(J + 1) * 128])
    ktri = ktri.astype(nd)

    alphaf = np.ascontiguousarray(
        alpha.astype(np.float32).reshape(NT, 128).T)        # [128, NT]

    shared = {"xa01": xa01, "xa2": xa2, "z2negh": z2negh, "ktri": ktri,
              "alphaf": alphaf}

    in_maps = []
    for b in range(B):
        Xb = X[b].astype(np.float32)                        # [M, P]
        xb01 = np.ascontiguousarray(
            Xb.T.reshape(2, 128, M).transpose(1, 0, 2)).astype(nd)  # [128, 2, M]
        ab = A[b].astype(np.float32)
        xb2 = np.empty((2, M), dtype=bf)
        xb2[0] = ab.astype(bf)
        xb2[1] = (-0.5 * (np.sum(Xb * Xb, axis=1) + ab)).astype(bf)
        in_maps.append({**shared, "xb01": xb01, "xb2": xb2})
    return in_maps


def _run(X, A, XA_train, alpha, K_inv, trace=False, tmpdir=None):
    from concourse.bass_utils import run_bass_kernel_spmd

    if "nc" not in _CACHE:
        _CACHE["nc"] = _build_program()
    nc = _CACHE["nc"]

    in_maps = _host_inputs(X, A, XA_train, alpha, K_inv)
    kw = {}
    if trace:
        kw = dict(trace=True, tmpdir=tmpdir)
    res = run_bass_kernel_spmd(nc, in_maps, core_ids=list(range(B)), **kw)

    f_loc = np.stack([res.results[b]["out"][0] for b in range(B)]).astype(np.float32)
    f_var = np.stack([res.results[b]["out"][1] for b in range(B)]).astype(np.float32)
    return (f_loc, f_var), res


def kernel(X, A, XA_train, alpha, K_inv):
    (f_loc, f_var), _ = _run(
        np.asarray(X), np.asarray(A), np.asarray(XA_train),
        np.asarray(alpha), np.asarray(K_inv),
    )
    return f_loc, f_var
